# revision 34
# baseline (speedup 1.0000x reference)
"""MixedExpertLayer Trainium2 kernel, v8: host-routed, globally balanced top-2 MoE.

Routing is resolved on the host: for each MLP expert e in {0,1} the tokens with
nonzero combined weight c_e = sum_k w_k*[idx_k==e] (~43.75% of all tokens) are
gathered into a compacted feature-major stream. The streams are split EVENLY
across the 8 cores (tokens have no core affinity for the MLP part), so every
core runs exactly ceil(total_e/8) ~ 900 tokens per expert instead of a padded
worst case. The device runs the SwiGLU MLP only on those columns:

  gate/up: psum[i-tile, tok] = sum_h W[h,i]^T x[h, tok]        (feature-major)
  A = silu(g)*u on ACT+DVE
  down (feature-major): psum[h-tile, tok] = sum_i Wd[i,h]^T A[i, tok]
  scaled by c_e via one DVE tensor-tensor with a host-broadcast coefficient row

Y_e is written feature-major [H, C_e]; the host transposes and scatter-adds.

Conv experts (2,3) run densely over each core's home token range on the PE as
diagonal-matrix matmuls (4 taps accumulated in PSUM, diag matrices built
on-device from an identity via ACT per-partition scaling), then ACT silu and a
feature-major weighted combine (in-place 2-operand DVE ops with host-broadcast
c2/c3 rows - these run at 1 elem/cycle unlike 3-operand ops). The conv partial
is stored feature-major [H, T]; host transposes and adds. No PE transposes.

Conv ops are fed through a drain queue that interleaves them between MLP
matmul groups, so no in-order engine stream is blocked by a long foreign op.
"""

import numpy as np
import ml_dtypes

import concourse.bass as bass
import concourse.mybir as mybir
import concourse.tile as tile
from concourse.bass_utils import run_bass_kernel_spmd
from concourse.masks import make_identity

B, S, H, I, KTOP, KC = 4, 4096, 1024, 2048, 2, 4
NCORES = 8
T = (B * S) // NCORES          # 2048 tokens per core
TH = T + KC - 1                # 2051 cols with halo
TCH = 512                      # token chunk (matmul N / PSUM bank)
NCHUNK = T // TCH              # 4 conv chunks
HK = H // 128                  # 8 h-chunks
IK = I // 128                  # 16 i-chunks
BF16 = mybir.dt.bfloat16
F32 = mybir.dt.float32
AF = mybir.ActivationFunctionType
MUL = mybir.AluOpType.mult
ADD = mybir.AluOpType.add

# routing state, set by build_in_maps (device capacities per MLP expert and
# per-(core, expert) global token lists)
_ROUTE = {"C": [1024, 1024], "lists": None}


def legalize_waits(nc):
    """This walrus build encodes exactly one sync-wait per instruction
    (single NEURON_ISA_TPB_EVENTS slot); Tile emits up to 3 plus a multi-wait
    tail Drain. Split extra waits onto wait-only EventSemaphore carriers
    inserted immediately before the instruction (same engine, same position,
    so no reordering and no deadlock risk)."""
    f = nc.m.functions[0]
    for blk in f.blocks:
        new = []
        for ins in list(blk.instructions):
            si = ins.sync_info
            if si is not None and si.on_wait and len(si.on_wait) > 1:
                best, order = {}, []
                for w in si.on_wait:
                    k = (w.sync_type, w.id, w.wait_mode)
                    if k not in best:
                        best[k] = w
                        order.append(k)
                    elif (w.wait_value or 0) > (best[k].wait_value or 0):
                        best[k] = w
                waits = [best[k] for k in order]
                for j, w in enumerate(waits[:-1]):
                    ev = mybir.InstEventSemaphore(
                        name=f"{ins.name}-lw{j}", engine=ins.engine, ins=[], outs=[],
                    )
                    ev.sync_info = mybir.SyncInfo(on_wait=[w], on_update=[])
                    new.append(ev)
                si.on_wait = [waits[-1]]
                ins.sync_info = si
            new.append(ins)
        blk.instructions = new
    return nc


def _windows(Ce):
    """Chunk windows (w0, n) covering Ce tokens in <=TCH pieces."""
    w, out = 0, []
    while w < Ce:
        n = min(TCH, Ce - w)
        out.append((w, n))
        w += n
    return out


def build_nc():
    C0, C1 = _ROUTE["C"]
    nc = bass.Bass(num_devices=NCORES)
    xf = nc.dram_tensor("xf", [H, TH], BF16, kind="ExternalInput")
    xg0 = nc.dram_tensor("xg0", [128, HK, C0], BF16, kind="ExternalInput")
    xg1 = nc.dram_tensor("xg1", [128, HK, C1], BF16, kind="ExternalInput")
    wgr = nc.dram_tensor("wgr", [2, IK, 128, HK, 128], BF16, kind="ExternalInput")
    wur = nc.dram_tensor("wur", [2, IK, 128, HK, 128], BF16, kind="ExternalInput")
    wdr = nc.dram_tensor("wdr", [2, IK, 128, HK, 128], BF16, kind="ExternalInput")
    cwp = nc.dram_tensor("cwp", [128, 2, HK, KC], F32, kind="ExternalInput")
    cgb0 = nc.dram_tensor("cgb0", [128, C0], BF16, kind="ExternalInput")
    cgb1 = nc.dram_tensor("cgb1", [128, C1], BF16, kind="ExternalInput")
    c23b = nc.dram_tensor("c23b", [128, 2, T], BF16, kind="ExternalInput")
    outc = nc.dram_tensor("outc", [H, T], BF16, kind="ExternalOutput")
    yf0 = nc.dram_tensor("yf0", [H, C0], BF16, kind="ExternalOutput")
    yf1 = nc.dram_tensor("yf1", [H, C1], BF16, kind="ExternalOutput")

    xf_t = xf.rearrange("(o p) t -> p o t", p=128)        # [128, HK, TH]
    outc_t = outc.rearrange("(o p) t -> p o t", p=128)    # [128, HK, T]
    yf_t = [y.rearrange("(o p) t -> p o t", p=128) for y in (yf0, yf1)]
    xg_d = [xg0, xg1]
    cgb_d = [cgb0, cgb1]

    phases = [(0, w0, n) for (w0, n) in _windows(C0)] + \
             [(1, w0, n) for (w0, n) in _windows(C1)]

    with tile.TileContext(nc) as tc:
        with (
            tc.tile_pool(name="singles", bufs=1) as singles,
            tc.tile_pool(name="wpool", bufs=6) as wpool,
            tc.tile_pool(name="wdpool", bufs=17) as wdpool,
            tc.tile_pool(name="apool", bufs=2) as apool,
            tc.tile_pool(name="spool", bufs=2) as spool,
            tc.tile_pool(name="ocp", bufs=1) as ocpool,
            tc.tile_pool(name="sgp", bufs=2) as sgpool,
            tc.tile_pool(name="ytp", bufs=3) as ytpool,
            tc.tile_pool(name="ps", bufs=2, space="PSUM") as ps,
            tc.tile_pool(name="pd", bufs=4, space="PSUM") as pd,
        ):
            # ---- phase-0-critical DMAs first: xg0, then small state ----
            xg_sb = [singles.tile([128, HK, Cx], BF16, name=f"xg{i}")
                     for i, Cx in enumerate((C0, C1))]

            def xg_load(e):
                for hk in range(HK):
                    nc.sync.dma_start(xg_sb[e][:, hk], xg_d[e][:, hk])

            xg_load(0)
            cw_sb = singles.tile([128, 2, HK, KC], F32)
            nc.sync.dma_start(cw_sb, cwp[:])
            cgb_sb = [singles.tile([128, Cx], BF16, name=f"cgb{i}")
                      for i, Cx in enumerate((C0, C1))]

            # conv diag matrices, built on-device: diag(cw[e2+e, hk*128+p, j])
            ident = singles.tile([128, 128], BF16)
            make_identity(nc, ident)
            diag_sb = singles.tile([128, 2, HK, KC, 128], BF16)
            for e in range(2):
                for hk in range(HK):
                    for j in range(KC):
                        nc.scalar.activation(
                            out=diag_sb[:, e, hk, j, :], in_=ident,
                            func=AF.Copy, scale=cw_sb[:, e, hk, j : j + 1])

            c23_sb = singles.tile([128, 2, T], BF16)
            xf_sb = singles.tile([128, HK, TH], BF16)

            # ---- conv experts: drain-queue units of (chunk, hk) ----
            def conv_unit(cc, hk, oc):
                t0 = cc * TCH
                psc = [None, None]
                s = [None, None]

                def mk_mm(e):
                    def mm(e=e):
                        psc[e] = ps.tile([128, TCH], F32,
                                         tag="pg" if e == 0 else "pu",
                                         name="psc")
                        for j in range(KC):
                            nc.tensor.matmul(
                                psc[e], diag_sb[:, e, hk, j, :],
                                xf_sb[:, hk, t0 + j : t0 + j + TCH],
                                start=(j == 0), stop=(j == KC - 1))
                    return mm

                def mk_silu(e):
                    def op(e=e):
                        s[e] = spool.tile([128, TCH], BF16,
                                          tag=f"s{e}", name="sconv")
                        nc.scalar.activation(out=s[e], in_=psc[e], func=AF.Silu)
                    return op

                def mk_mul(e):
                    def op(e=e):
                        nc.vector.tensor_mul(
                            s[e], s[e], c23_sb[:, e, t0 : t0 + TCH])
                    return op

                def mk_add():
                    def op():
                        nc.vector.tensor_add(oc[:, hk, :], s[0], s[1])
                    return op

                return [mk_mm(0), mk_silu(0), mk_mm(1), mk_silu(1),
                        mk_mul(0), mk_mul(1), mk_add()]

            pending = [
                lambda: nc.sync.dma_start(c23_sb[:, 0], c23b[:, 0]),
                lambda: nc.sync.dma_start(c23_sb[:, 1], c23b[:, 1]),
                lambda: nc.sync.dma_start(cgb_sb[0], cgb_d[0][:]),
                lambda: nc.sync.dma_start(cgb_sb[1], cgb_d[1][:]),
            ]
            for hk in range(HK):
                pending.append(
                    lambda hk=hk: nc.sync.dma_start(xf_sb[:, hk], xf_t[:, hk]))

            conv_left = [(cc, hk) for cc in range(NCHUNK) for hk in range(HK)]
            oc_tiles = {}

            def next_unit():
                cc, hk = conv_left.pop(0)
                if hk == 0:
                    oc_tiles[cc] = ocpool.tile([128, HK, TCH], BF16, tag="oc",
                                               name="oc")
                ops = conv_unit(cc, hk, oc_tiles[cc])
                if hk == HK - 1:
                    oc = oc_tiles.pop(cc)

                    def store(cc=cc, oc=oc):
                        nc.sync.dma_start(
                            outc_t[:, :, cc * TCH : (cc + 1) * TCH], oc)
                    ops.append(store)
                return ops

            nslots = sum(IK + HK for _ in phases)
            nops = 9 + NCHUNK * (HK * 7 + 1)
            dn = max(2, -(-nops // max(nslots - 12, 1)))

            slot_idx = [0]

            def drain(k):
                # during warmup only the preamble DMAs drain, so the first
                # conv matmuls (which wait on xf) don't block the PE stream
                slot_idx[0] += 1
                for _ in range(k):
                    if not pending:
                        if not conv_left or slot_idx[0] <= 12:
                            return
                        pending.extend(next_unit())
                    pending.pop(0)()

            for pi, (e, w0, nw) in enumerate(phases):
                # ---- gate/up -> A (feature-major [I, nw]) ----
                a_sb = apool.tile([128, IK, TCH], BF16, tag="a")
                wds = []
                for i in range(IK):
                    # weight tiles via the ACT hwdge queue: descriptor-gen in
                    # parallel with the SP engine's other DMA streams
                    wgt = wpool.tile([128, HK, 128], BF16, tag="wg")
                    nc.scalar.dma_start(wgt, wgr[e, i])
                    wut = wpool.tile([128, HK, 128], BF16, tag="wu")
                    nc.scalar.dma_start(wut, wur[e, i])
                    psg = ps.tile([128, TCH], F32, tag="pg")
                    psu = ps.tile([128, TCH], F32, tag="pu")
                    for kc in range(HK):
                        nc.tensor.matmul(
                            psg[:, 0:nw], wgt[:, kc, :],
                            xg_sb[e][:, kc, w0 : w0 + nw],
                            start=(kc == 0), stop=(kc == HK - 1))
                    for kc in range(HK):
                        nc.tensor.matmul(
                            psu[:, 0:nw], wut[:, kc, :],
                            xg_sb[e][:, kc, w0 : w0 + nw],
                            start=(kc == 0), stop=(kc == HK - 1))
                    sg = sgpool.tile([128, TCH], F32, tag="sg")
                    nc.scalar.activation(
                        out=sg[:, 0:nw], in_=psg[:, 0:nw], func=AF.Silu)
                    nc.vector.tensor_mul(
                        a_sb[:, i, 0:nw], sg[:, 0:nw], psu[:, 0:nw])
                    if i == 8:
                        # mid-phase: prefetch down weights + next xg stream
                        for kc in range(IK):
                            wdt = wdpool.tile([128, HK, 128], BF16, tag="wd",
                                              name="wdt")
                            nc.sync.dma_start(wdt, wdr[e, kc])
                            wds.append(wdt)
                        if pi == 0:
                            xg_load(1)
                    drain(dn)

                # ---- down, feature-major: psum[h-tile, tok] ----
                for hb in range(HK):
                    psd = pd.tile([128, TCH], F32, tag="pd")
                    for kc in range(IK):
                        nc.tensor.matmul(
                            psd[:, 0:nw], wds[kc][:, hb, :],
                            a_sb[:, kc, 0:nw],
                            start=(kc == 0), stop=(kc == IK - 1))
                    yt = ytpool.tile([128, TCH], BF16, tag="yt")
                    nc.vector.tensor_mul(
                        yt[:, 0:nw], psd[:, 0:nw],
                        cgb_sb[e][:, w0 : w0 + nw])
                    if pi == len(phases) - 1:
                        # last phase: split the store for lower tail latency
                        h2 = nw // 2
                        nc.sync.dma_start(
                            yf_t[e][:, hb, w0 : w0 + h2], yt[:, 0:h2])
                        nc.sync.dma_start(
                            yf_t[e][:, hb, w0 + h2 : w0 + nw], yt[:, h2:nw])
                    else:
                        nc.sync.dma_start(
                            yf_t[e][:, hb, w0 : w0 + nw], yt[:, 0:nw])
                    drain(dn)
            # flush any remaining conv work
            while pending or conv_left:
                drain(16)
    return legalize_waits(nc)


def _bf16(a):
    return np.asarray(a).astype(ml_dtypes.bfloat16)


def build_in_maps(x, top_k_indices, norm_weights, mlp_gate, mlp_up, mlp_down, conv_w):
    NT = B * S
    xflat = np.asarray(x, dtype=np.float32).reshape(NT, H)
    idxflat = np.asarray(top_k_indices).reshape(NT, KTOP)
    nwflat = np.asarray(norm_weights, dtype=np.float32).reshape(NT, KTOP)

    # combined per-expert coefficients, global
    ce = np.zeros((NT, 4), dtype=np.float32)
    rows = np.arange(NT)
    for k in range(KTOP):
        np.add.at(ce, (rows, idxflat[:, k]), nwflat[:, k])

    # globally balanced routing: split each expert's token list evenly
    lists = [[], []]
    Cs = [0, 0]
    for e in range(2):
        glst = np.nonzero(ce[:, e] != 0.0)[0]
        lists[e] = np.array_split(glst, NCORES)
        Cs[e] = max(len(l) for l in lists[e])
    _ROUTE["C"] = Cs
    _ROUTE["lists"] = lists

    # weights, repacked so every DMA tile is contiguous per partition
    wgr = np.ascontiguousarray(
        _bf16(mlp_gate).reshape(2, HK, 128, IK, 128).transpose(0, 3, 2, 1, 4))
    wur = np.ascontiguousarray(
        _bf16(mlp_up).reshape(2, HK, 128, IK, 128).transpose(0, 3, 2, 1, 4))
    wdr = np.ascontiguousarray(_bf16(mlp_down).reshape(2, IK, 128, HK, 128))
    cw = np.asarray(conv_w, dtype=np.float32).reshape(2, HK, 128, KC)
    cwp = np.ascontiguousarray(cw.transpose(2, 0, 1, 3))  # [128, 2, HK, KC]

    in_maps = []
    for i in range(NCORES):
        lo = i * T
        if i % 2 == 0:
            halo = np.zeros((KC - 1, H), dtype=np.float32)
        else:
            halo = xflat[lo - (KC - 1) : lo]
        xh = np.concatenate([halo, xflat[lo : lo + T]], axis=0)  # [T+3, H]
        xf = np.ascontiguousarray(_bf16(xh).T)                   # [H, T+3]

        im = {"xf": xf, "wgr": wgr, "wur": wur, "wdr": wdr, "cwp": cwp}
        for e in range(2):
            lst = lists[e][i]
            n = len(lst)
            Cx = Cs[e]
            xg = np.zeros((H, Cx), dtype=ml_dtypes.bfloat16)
            xg[:, :n] = _bf16(xflat[lst]).T
            im[f"xg{e}"] = np.ascontiguousarray(
                xg.reshape(HK, 128, Cx).transpose(1, 0, 2))
            cvec = np.zeros(Cx, dtype=np.float32)
            cvec[:n] = ce[lst, e]
            im[f"cgb{e}"] = np.ascontiguousarray(
                np.broadcast_to(cvec[None, :], (128, Cx))).astype(
                    ml_dtypes.bfloat16)
        im["c23b"] = np.ascontiguousarray(
            np.broadcast_to(ce[lo : lo + T, 2:4].T[:, None, :], (2, 128, T))
            .transpose(1, 0, 2)).astype(ml_dtypes.bfloat16)
        in_maps.append(im)
    return in_maps


def assemble(results):
    lists = _ROUTE["lists"]
    out = np.empty((NT_G := B * S, H), dtype=np.float32)
    for i, r in enumerate(results):
        out[i * T : (i + 1) * T] = np.asarray(r["outc"], dtype=np.float32).T
    for i, r in enumerate(results):
        for e in range(2):
            lst = lists[e][i]
            n = len(lst)
            yv = np.asarray(r[f"yf{e}"], dtype=np.float32)  # [H, C_e]
            out[lst] += yv[:, :n].T
    return out.reshape(B, S, H)


def kernel(x, top_k_indices, norm_weights, mlp_gate, mlp_up, mlp_down, conv_w):
    in_maps = build_in_maps(
        x, top_k_indices, norm_weights, mlp_gate, mlp_up, mlp_down, conv_w
    )
    nc = build_nc()
    res = run_bass_kernel_spmd(nc, in_maps, core_ids=list(range(NCORES)))
    return assemble(res.results)


# revision 35
# speedup vs baseline: 1.2124x; 1.2124x over previous
"""MixedExpertLayer Trainium2 kernel, v8: host-routed, globally balanced top-2 MoE.

Routing is resolved on the host: for each MLP expert e in {0,1} the tokens with
nonzero combined weight c_e = sum_k w_k*[idx_k==e] (~43.75% of all tokens) are
gathered into a compacted feature-major stream. The streams are split EVENLY
across the 8 cores (tokens have no core affinity for the MLP part), so every
core runs exactly ceil(total_e/8) ~ 900 tokens per expert instead of a padded
worst case. The device runs the SwiGLU MLP only on those columns:

  gate/up: psum[i-tile, tok] = sum_h W[h,i]^T x[h, tok]        (feature-major)
  A = silu(g)*u on ACT+DVE
  down (feature-major): psum[h-tile, tok] = sum_i Wd[i,h]^T A[i, tok]
  scaled by c_e via one DVE tensor-tensor with a host-broadcast coefficient row

Y_e is written feature-major [H, C_e]; the host transposes and scatter-adds.

Conv experts (2,3) run densely over each core's home token range on the PE as
diagonal-matrix matmuls (4 taps accumulated in PSUM, diag matrices built
on-device from an identity via ACT per-partition scaling), then ACT silu and a
feature-major weighted combine (in-place 2-operand DVE ops with host-broadcast
c2/c3 rows - these run at 1 elem/cycle unlike 3-operand ops). The conv partial
is stored feature-major [H, T]; host transposes and adds. No PE transposes.

Conv ops are fed through a drain queue that interleaves them between MLP
matmul groups, so no in-order engine stream is blocked by a long foreign op.
"""

import numpy as np
import ml_dtypes

import concourse.bass as bass
import concourse.mybir as mybir
import concourse.tile as tile
from concourse.bass_utils import run_bass_kernel_spmd
from concourse.masks import make_identity

B, S, H, I, KTOP, KC = 4, 4096, 1024, 2048, 2, 4
NCORES = 8
T = (B * S) // NCORES          # 2048 tokens per core
TH = T + KC - 1                # 2051 cols with halo
TCH = 512                      # token chunk (matmul N / PSUM bank)
NCHUNK = T // TCH              # 4 conv chunks
HK = H // 128                  # 8 h-chunks
IK = I // 128                  # 16 i-chunks
BF16 = mybir.dt.bfloat16
F32 = mybir.dt.float32
AF = mybir.ActivationFunctionType
MUL = mybir.AluOpType.mult
ADD = mybir.AluOpType.add

# routing state, set by build_in_maps (device capacities per MLP expert and
# per-(core, expert) global token lists)
_ROUTE = {"C": [1024, 1024], "lists": None}


def legalize_waits(nc):
    """This walrus build encodes exactly one sync-wait per instruction
    (single NEURON_ISA_TPB_EVENTS slot); Tile emits up to 3 plus a multi-wait
    tail Drain. Split extra waits onto wait-only EventSemaphore carriers
    inserted immediately before the instruction (same engine, same position,
    so no reordering and no deadlock risk)."""
    f = nc.m.functions[0]
    for blk in f.blocks:
        new = []
        for ins in list(blk.instructions):
            si = ins.sync_info
            if si is not None and si.on_wait and len(si.on_wait) > 1:
                best, order = {}, []
                for w in si.on_wait:
                    k = (w.sync_type, w.id, w.wait_mode)
                    if k not in best:
                        best[k] = w
                        order.append(k)
                    elif (w.wait_value or 0) > (best[k].wait_value or 0):
                        best[k] = w
                waits = [best[k] for k in order]
                for j, w in enumerate(waits[:-1]):
                    ev = mybir.InstEventSemaphore(
                        name=f"{ins.name}-lw{j}", engine=ins.engine, ins=[], outs=[],
                    )
                    ev.sync_info = mybir.SyncInfo(on_wait=[w], on_update=[])
                    new.append(ev)
                si.on_wait = [waits[-1]]
                ins.sync_info = si
            new.append(ins)
        blk.instructions = new
    return nc


def _windows(Ce):
    """Chunk windows (w0, n) covering Ce tokens in <=TCH pieces."""
    w, out = 0, []
    while w < Ce:
        n = min(TCH, Ce - w)
        out.append((w, n))
        w += n
    return out


def build_nc():
    C0, C1 = _ROUTE["C"]
    nc = bass.Bass(num_devices=NCORES)
    xf = nc.dram_tensor("xf", [H, TH], BF16, kind="ExternalInput")
    xg0 = nc.dram_tensor("xg0", [128, HK, C0], BF16, kind="ExternalInput")
    xg1 = nc.dram_tensor("xg1", [128, HK, C1], BF16, kind="ExternalInput")
    wgr = nc.dram_tensor("wgr", [2, IK, 128, HK, 128], BF16, kind="ExternalInput")
    wur = nc.dram_tensor("wur", [2, IK, 128, HK, 128], BF16, kind="ExternalInput")
    wdr = nc.dram_tensor("wdr", [2, IK, 128, HK, 128], BF16, kind="ExternalInput")
    cwp = nc.dram_tensor("cwp", [128, 2, HK, KC], F32, kind="ExternalInput")
    cgb0 = nc.dram_tensor("cgb0", [128, C0], BF16, kind="ExternalInput")
    cgb1 = nc.dram_tensor("cgb1", [128, C1], BF16, kind="ExternalInput")
    c23b = nc.dram_tensor("c23b", [128, 2, T], BF16, kind="ExternalInput")
    outc = nc.dram_tensor("outc", [H, T], BF16, kind="ExternalOutput")
    yf0 = nc.dram_tensor("yf0", [H, C0], BF16, kind="ExternalOutput")
    yf1 = nc.dram_tensor("yf1", [H, C1], BF16, kind="ExternalOutput")

    xf_t = xf.rearrange("(o p) t -> p o t", p=128)        # [128, HK, TH]
    outc_t = outc.rearrange("(o p) t -> p o t", p=128)    # [128, HK, T]
    yf_t = [y.rearrange("(o p) t -> p o t", p=128) for y in (yf0, yf1)]
    xg_d = [xg0, xg1]
    cgb_d = [cgb0, cgb1]

    phases = [(0, w0, n) for (w0, n) in _windows(C0)] + \
             [(1, w0, n) for (w0, n) in _windows(C1)]

    with tile.TileContext(nc) as tc:
        with (
            tc.tile_pool(name="singles", bufs=1) as singles,
            tc.tile_pool(name="wpool", bufs=6) as wpool,
            tc.tile_pool(name="wdpool", bufs=17) as wdpool,
            tc.tile_pool(name="apool", bufs=2) as apool,
            tc.tile_pool(name="spool", bufs=2) as spool,
            tc.tile_pool(name="ocp", bufs=1) as ocpool,
            tc.tile_pool(name="sgp", bufs=2) as sgpool,
            tc.tile_pool(name="ytp", bufs=3) as ytpool,
            tc.tile_pool(name="ps", bufs=2, space="PSUM") as ps,
            tc.tile_pool(name="pd", bufs=4, space="PSUM") as pd,
        ):
            # ---- phase-0-critical DMAs first: xg0, then small state ----
            xg_sb = [singles.tile([128, HK, Cx], BF16, name=f"xg{i}")
                     for i, Cx in enumerate((C0, C1))]

            def xg_load(e):
                for hk in range(HK):
                    nc.sync.dma_start(xg_sb[e][:, hk], xg_d[e][:, hk])

            xg_load(0)
            cw_sb = singles.tile([128, 2, HK, KC], F32)
            nc.sync.dma_start(cw_sb, cwp[:])
            cgb_sb = [singles.tile([128, Cx], BF16, name=f"cgb{i}")
                      for i, Cx in enumerate((C0, C1))]

            # conv diag matrices, built on-device: diag(cw[e2+e, hk*128+p, j])
            ident = singles.tile([128, 128], BF16)
            make_identity(nc, ident)
            diag_sb = singles.tile([128, 2, HK, KC, 128], BF16)
            for e in range(2):
                for hk in range(HK):
                    for j in range(KC):
                        nc.scalar.activation(
                            out=diag_sb[:, e, hk, j, :], in_=ident,
                            func=AF.Copy, scale=cw_sb[:, e, hk, j : j + 1])

            c23_sb = singles.tile([128, 2, T], BF16)
            xf_sb = singles.tile([128, HK, TH], BF16)

            # ---- conv experts: drain-queue units of (chunk, hk) ----
            def conv_unit(cc, hk, oc):
                t0 = cc * TCH
                psc = [None, None]
                s = [None, None]

                def mk_mm(e):
                    def mm(e=e):
                        psc[e] = ps.tile([128, TCH], F32,
                                         tag="pg" if e == 0 else "pu",
                                         name="psc")
                        for j in range(KC):
                            nc.tensor.matmul(
                                psc[e], diag_sb[:, e, hk, j, :],
                                xf_sb[:, hk, t0 + j : t0 + j + TCH],
                                start=(j == 0), stop=(j == KC - 1))
                    return mm

                def mk_silu(e):
                    def op(e=e):
                        s[e] = spool.tile([128, TCH], BF16,
                                          tag=f"s{e}", name="sconv")
                        nc.scalar.activation(out=s[e], in_=psc[e], func=AF.Silu)
                    return op

                def mk_mul(e):
                    def op(e=e):
                        nc.vector.tensor_mul(
                            s[e], s[e], c23_sb[:, e, t0 : t0 + TCH])
                    return op

                def mk_add():
                    def op():
                        nc.vector.tensor_add(oc[:, hk, :], s[0], s[1])
                    return op

                return [mk_mm(0), mk_silu(0), mk_mm(1), mk_silu(1),
                        mk_mul(0), mk_mul(1), mk_add()]

            pending = [
                lambda: nc.sync.dma_start(c23_sb[:, 0], c23b[:, 0]),
                lambda: nc.sync.dma_start(c23_sb[:, 1], c23b[:, 1]),
                lambda: nc.sync.dma_start(cgb_sb[0], cgb_d[0][:]),
                lambda: nc.sync.dma_start(cgb_sb[1], cgb_d[1][:]),
            ]
            for hk in range(HK):
                pending.append(
                    lambda hk=hk: nc.sync.dma_start(xf_sb[:, hk], xf_t[:, hk]))

            conv_left = [(cc, hk) for cc in range(NCHUNK) for hk in range(HK)]
            oc_tiles = {}

            def next_unit():
                cc, hk = conv_left.pop(0)
                if hk == 0:
                    oc_tiles[cc] = ocpool.tile([128, HK, TCH], BF16, tag="oc",
                                               name="oc")
                ops = conv_unit(cc, hk, oc_tiles[cc])
                if hk == HK - 1:
                    oc = oc_tiles.pop(cc)

                    def store(cc=cc, oc=oc):
                        nc.sync.dma_start(
                            outc_t[:, :, cc * TCH : (cc + 1) * TCH], oc)
                    ops.append(store)
                return ops

            nslots = sum(IK + HK for _ in phases)
            nops = 9 + NCHUNK * (HK * 7 + 1)
            dn = max(2, -(-nops // max(nslots - 12, 1)))

            slot_idx = [0]

            def drain(k):
                # during warmup only the preamble DMAs drain, so the first
                # conv matmuls (which wait on xf) don't block the PE stream
                slot_idx[0] += 1
                for _ in range(k):
                    if not pending:
                        if not conv_left or slot_idx[0] <= 12:
                            return
                        pending.extend(next_unit())
                    pending.pop(0)()

            for pi, (e, w0, nw) in enumerate(phases):
                # ---- gate/up -> A (feature-major [I, nw]) ----
                a_sb = apool.tile([128, IK, TCH], BF16, tag="a")
                wds = []
                for i in range(IK):
                    wgt = wpool.tile([128, HK, 128], BF16, tag="wg")
                    nc.sync.dma_start(wgt, wgr[e, i])
                    wut = wpool.tile([128, HK, 128], BF16, tag="wu")
                    nc.sync.dma_start(wut, wur[e, i])
                    psg = ps.tile([128, TCH], F32, tag="pg")
                    psu = ps.tile([128, TCH], F32, tag="pu")
                    for kc in range(HK):
                        nc.tensor.matmul(
                            psg[:, 0:nw], wgt[:, kc, :],
                            xg_sb[e][:, kc, w0 : w0 + nw],
                            start=(kc == 0), stop=(kc == HK - 1))
                    for kc in range(HK):
                        nc.tensor.matmul(
                            psu[:, 0:nw], wut[:, kc, :],
                            xg_sb[e][:, kc, w0 : w0 + nw],
                            start=(kc == 0), stop=(kc == HK - 1))
                    sg = sgpool.tile([128, TCH], F32, tag="sg")
                    nc.scalar.activation(
                        out=sg[:, 0:nw], in_=psg[:, 0:nw], func=AF.Silu)
                    nc.vector.tensor_mul(
                        a_sb[:, i, 0:nw], sg[:, 0:nw], psu[:, 0:nw])
                    if i == 8:
                        # mid-phase: prefetch down weights + next xg stream
                        for kc in range(IK):
                            wdt = wdpool.tile([128, HK, 128], BF16, tag="wd",
                                              name="wdt")
                            nc.sync.dma_start(wdt, wdr[e, kc])
                            wds.append(wdt)
                        if pi == 0:
                            xg_load(1)
                    drain(dn)

                # ---- down, feature-major: psum[h-tile, tok] ----
                for hb in range(HK):
                    psd = pd.tile([128, TCH], F32, tag="pd")
                    for kc in range(IK):
                        nc.tensor.matmul(
                            psd[:, 0:nw], wds[kc][:, hb, :],
                            a_sb[:, kc, 0:nw],
                            start=(kc == 0), stop=(kc == IK - 1))
                    yt = ytpool.tile([128, TCH], BF16, tag="yt")
                    nc.vector.tensor_mul(
                        yt[:, 0:nw], psd[:, 0:nw],
                        cgb_sb[e][:, w0 : w0 + nw])
                    if pi == len(phases) - 1:
                        # last phase: split the store for lower tail latency
                        h2 = nw // 2
                        nc.sync.dma_start(
                            yf_t[e][:, hb, w0 : w0 + h2], yt[:, 0:h2])
                        nc.sync.dma_start(
                            yf_t[e][:, hb, w0 + h2 : w0 + nw], yt[:, h2:nw])
                    else:
                        nc.sync.dma_start(
                            yf_t[e][:, hb, w0 : w0 + nw], yt[:, 0:nw])
                    drain(dn)
            # flush any remaining conv work
            while pending or conv_left:
                drain(16)
    return legalize_waits(nc)


def _bf16(a):
    return np.asarray(a).astype(ml_dtypes.bfloat16)


def build_in_maps(x, top_k_indices, norm_weights, mlp_gate, mlp_up, mlp_down, conv_w):
    NT = B * S
    xflat = np.asarray(x, dtype=np.float32).reshape(NT, H)
    idxflat = np.asarray(top_k_indices).reshape(NT, KTOP)
    nwflat = np.asarray(norm_weights, dtype=np.float32).reshape(NT, KTOP)

    # combined per-expert coefficients, global
    ce = np.zeros((NT, 4), dtype=np.float32)
    rows = np.arange(NT)
    for k in range(KTOP):
        np.add.at(ce, (rows, idxflat[:, k]), nwflat[:, k])

    # globally balanced routing: split each expert's token list evenly
    lists = [[], []]
    Cs = [0, 0]
    for e in range(2):
        glst = np.nonzero(ce[:, e] != 0.0)[0]
        lists[e] = np.array_split(glst, NCORES)
        Cs[e] = max(len(l) for l in lists[e])
    _ROUTE["C"] = Cs
    _ROUTE["lists"] = lists

    # weights, repacked so every DMA tile is contiguous per partition
    wgr = np.ascontiguousarray(
        _bf16(mlp_gate).reshape(2, HK, 128, IK, 128).transpose(0, 3, 2, 1, 4))
    wur = np.ascontiguousarray(
        _bf16(mlp_up).reshape(2, HK, 128, IK, 128).transpose(0, 3, 2, 1, 4))
    wdr = np.ascontiguousarray(_bf16(mlp_down).reshape(2, IK, 128, HK, 128))
    cw = np.asarray(conv_w, dtype=np.float32).reshape(2, HK, 128, KC)
    cwp = np.ascontiguousarray(cw.transpose(2, 0, 1, 3))  # [128, 2, HK, KC]

    in_maps = []
    for i in range(NCORES):
        lo = i * T
        if i % 2 == 0:
            halo = np.zeros((KC - 1, H), dtype=np.float32)
        else:
            halo = xflat[lo - (KC - 1) : lo]
        xh = np.concatenate([halo, xflat[lo : lo + T]], axis=0)  # [T+3, H]
        xf = np.ascontiguousarray(_bf16(xh).T)                   # [H, T+3]

        im = {"xf": xf, "wgr": wgr, "wur": wur, "wdr": wdr, "cwp": cwp}
        for e in range(2):
            lst = lists[e][i]
            n = len(lst)
            Cx = Cs[e]
            xg = np.zeros((H, Cx), dtype=ml_dtypes.bfloat16)
            xg[:, :n] = _bf16(xflat[lst]).T
            im[f"xg{e}"] = np.ascontiguousarray(
                xg.reshape(HK, 128, Cx).transpose(1, 0, 2))
            cvec = np.zeros(Cx, dtype=np.float32)
            cvec[:n] = ce[lst, e]
            im[f"cgb{e}"] = np.ascontiguousarray(
                np.broadcast_to(cvec[None, :], (128, Cx))).astype(
                    ml_dtypes.bfloat16)
        im["c23b"] = np.ascontiguousarray(
            np.broadcast_to(ce[lo : lo + T, 2:4].T[:, None, :], (2, 128, T))
            .transpose(1, 0, 2)).astype(ml_dtypes.bfloat16)
        in_maps.append(im)
    return in_maps


def assemble(results):
    lists = _ROUTE["lists"]
    out = np.empty((NT_G := B * S, H), dtype=np.float32)
    for i, r in enumerate(results):
        out[i * T : (i + 1) * T] = np.asarray(r["outc"], dtype=np.float32).T
    for i, r in enumerate(results):
        for e in range(2):
            lst = lists[e][i]
            n = len(lst)
            yv = np.asarray(r[f"yf{e}"], dtype=np.float32)  # [H, C_e]
            out[lst] += yv[:, :n].T
    return out.reshape(B, S, H)


def kernel(x, top_k_indices, norm_weights, mlp_gate, mlp_up, mlp_down, conv_w):
    in_maps = build_in_maps(
        x, top_k_indices, norm_weights, mlp_gate, mlp_up, mlp_down, conv_w
    )
    nc = build_nc()
    res = run_bass_kernel_spmd(nc, in_maps, core_ids=list(range(NCORES)))
    return assemble(res.results)


# revision 39
# speedup vs baseline: 1.3166x; 1.0859x over previous
"""MixedExpertLayer Trainium2 kernel, v11: fully routed, globally balanced MoE.

ALL four experts are routed on the host. For each expert the tokens with
nonzero combined weight c_e = sum_k w_k*[idx_k==e] (~43.75% of tokens) are
gathered into a compacted feature-major stream and split EVENLY across the 8
cores (no core affinity), so each core processes ~900 tokens per expert.

MLP experts 0,1 (per core, per expert, windows of <=512 tokens):
  gate/up: psum[i-tile, tok] = sum_h W[h,i]^T x[h, tok]   (feature-major)
  A = silu(g)*u on ACT+DVE
  down (feature-major): psum[h-tile, tok] = sum_i Wd[i,h]^T A[i, tok]
  scaled by c_e via one DVE mul with a host-broadcast coefficient row.

Conv experts 2,3: the host gathers FOUR tap-shifted copies of the selected
tokens (x[t-3+j] for j=0..3, zero at sequence starts), so the depthwise conv
becomes 4 accumulated diag-matrix matmuls on the PE over compacted columns
(diag(w_j) built on-device from an identity via ACT per-partition scaling),
then ACT silu and a DVE mul by the broadcast c_e row.

All outputs are compacted feature-major [H, C_e]; the host transposes and
scatter-adds the four streams into the zero-initialized result (fp32).

Conv ops are fed through a drain queue that interleaves them between MLP
matmul groups so no in-order engine stream is head-of-line blocked.
"""

import numpy as np
import ml_dtypes

import concourse.bass as bass
import concourse.mybir as mybir
import concourse.tile as tile
from concourse.bass_utils import run_bass_kernel_spmd
from concourse.masks import make_identity

B, S, H, I, KTOP, KC = 4, 4096, 1024, 2048, 2, 4
NCORES = 8
T = (B * S) // NCORES          # 2048 tokens per core
TCH = 512                      # token chunk (matmul N / PSUM bank)
HK = H // 128                  # 8 h-chunks
IK = I // 128                  # 16 i-chunks
BF16 = mybir.dt.bfloat16
F32 = mybir.dt.float32
AF = mybir.ActivationFunctionType
MUL = mybir.AluOpType.mult
ADD = mybir.AluOpType.add

# routing state set by build_in_maps: per-expert device capacities and
# per-(expert, core) global token lists
_ROUTE = {"C": [1024, 1024, 1024, 1024], "lists": None}


def legalize_waits(nc):
    """This walrus build encodes exactly one sync-wait per instruction
    (single NEURON_ISA_TPB_EVENTS slot); Tile emits up to 3 plus a multi-wait
    tail Drain. Split extra waits onto wait-only EventSemaphore carriers
    inserted immediately before the instruction (same engine, same position,
    so no reordering and no deadlock risk)."""
    f = nc.m.functions[0]
    for blk in f.blocks:
        new = []
        for ins in list(blk.instructions):
            si = ins.sync_info
            if si is not None and si.on_wait and len(si.on_wait) > 1:
                best, order = {}, []
                for w in si.on_wait:
                    k = (w.sync_type, w.id, w.wait_mode)
                    if k not in best:
                        best[k] = w
                        order.append(k)
                    elif (w.wait_value or 0) > (best[k].wait_value or 0):
                        best[k] = w
                waits = [best[k] for k in order]
                for j, w in enumerate(waits[:-1]):
                    ev = mybir.InstEventSemaphore(
                        name=f"{ins.name}-lw{j}", engine=ins.engine, ins=[], outs=[],
                    )
                    ev.sync_info = mybir.SyncInfo(on_wait=[w], on_update=[])
                    new.append(ev)
                si.on_wait = [waits[-1]]
                ins.sync_info = si
            new.append(ins)
        blk.instructions = new
    return nc


def _windows(Ce):
    """Chunk windows (w0, n) covering Ce tokens in <=TCH pieces."""
    w, out = 0, []
    while w < Ce:
        n = min(TCH, Ce - w)
        out.append((w, n))
        w += n
    return out


def build_nc():
    C0, C1, C2, C3 = _ROUTE["C"]
    nc = bass.Bass(num_devices=NCORES)
    xg0 = nc.dram_tensor("xg0", [128, HK, C0], BF16, kind="ExternalInput")
    xg1 = nc.dram_tensor("xg1", [128, HK, C1], BF16, kind="ExternalInput")
    xc0 = nc.dram_tensor("xc0", [128, HK, KC, C2], BF16, kind="ExternalInput")
    xc1 = nc.dram_tensor("xc1", [128, HK, KC, C3], BF16, kind="ExternalInput")
    wgr = nc.dram_tensor("wgr", [2, IK, 128, HK, 128], BF16, kind="ExternalInput")
    wur = nc.dram_tensor("wur", [2, IK, 128, HK, 128], BF16, kind="ExternalInput")
    wdr = nc.dram_tensor("wdr", [2, IK, 128, HK, 128], BF16, kind="ExternalInput")
    cwp = nc.dram_tensor("cwp", [128, 2, HK, KC], F32, kind="ExternalInput")
    cgb0 = nc.dram_tensor("cgb0", [128, C0], BF16, kind="ExternalInput")
    cgb1 = nc.dram_tensor("cgb1", [128, C1], BF16, kind="ExternalInput")
    cgc0 = nc.dram_tensor("cgc0", [128, C2], BF16, kind="ExternalInput")
    cgc1 = nc.dram_tensor("cgc1", [128, C3], BF16, kind="ExternalInput")
    yf0 = nc.dram_tensor("yf0", [H, C0], BF16, kind="ExternalOutput")
    yf1 = nc.dram_tensor("yf1", [H, C1], BF16, kind="ExternalOutput")
    yc0 = nc.dram_tensor("yc0", [H, C2], BF16, kind="ExternalOutput")
    yc1 = nc.dram_tensor("yc1", [H, C3], BF16, kind="ExternalOutput")

    yf_t = [y.rearrange("(o p) t -> p o t", p=128) for y in (yf0, yf1)]
    yc_t = [y.rearrange("(o p) t -> p o t", p=128) for y in (yc0, yc1)]
    xg_d = [xg0, xg1]
    xc_d = [xc0, xc1]
    cgb_d = [cgb0, cgb1]
    cgc_d = [cgc0, cgc1]
    Cconv = [C2, C3]

    phases = [(0, w0, n) for (w0, n) in _windows(C0)] + \
             [(1, w0, n) for (w0, n) in _windows(C1)]

    with tile.TileContext(nc) as tc:
        with (
            tc.tile_pool(name="singles", bufs=1) as singles,
            tc.tile_pool(name="wpool", bufs=6) as wpool,
            tc.tile_pool(name="wdpool", bufs=17) as wdpool,
            tc.tile_pool(name="apool", bufs=2) as apool,
            tc.tile_pool(name="xcpool", bufs=3) as xcpool,
            tc.tile_pool(name="spool", bufs=3) as spool,
            tc.tile_pool(name="sgp", bufs=2) as sgpool,
            tc.tile_pool(name="ytp", bufs=3) as ytpool,
            tc.tile_pool(name="ps", bufs=2, space="PSUM") as ps,
            tc.tile_pool(name="pd", bufs=4, space="PSUM") as pd,
        ):
            # ---- phase-0-critical DMAs first: xg0, then small state ----
            xg_sb = [singles.tile([128, HK, Cx], BF16, name=f"xg{i}")
                     for i, Cx in enumerate((C0, C1))]

            def xg_load(e):
                for hk in range(HK):
                    nc.sync.dma_start(xg_sb[e][:, hk], xg_d[e][:, hk])

            xg_load(0)
            cw_sb = singles.tile([128, 2, HK, KC], F32)
            nc.sync.dma_start(cw_sb, cwp[:])
            cgb_sb = [singles.tile([128, Cx], BF16, name=f"cgb{i}")
                      for i, Cx in enumerate((C0, C1))]
            cgc_sb = [singles.tile([128, Cx], BF16, name=f"cgc{i}")
                      for i, Cx in enumerate((C2, C3))]

            # conv diag matrices, built on-device: diag(cw[2+e, hk*128+p, j])
            ident = singles.tile([128, 128], BF16)
            make_identity(nc, ident)
            diag_sb = singles.tile([128, 2, HK, KC, 128], BF16)
            for e in range(2):
                for hk in range(HK):
                    for j in range(KC):
                        nc.scalar.activation(
                            out=diag_sb[:, e, hk, j, :], in_=ident,
                            func=AF.Copy, scale=cw_sb[:, e, hk, j : j + 1])

            # ---- conv experts: drain-queue units of (e, hk) ----
            xc_tiles = {}

            def xc_fetch(u):
                if u >= 2 * HK:
                    return
                e, hk = divmod(u, HK)
                xct = xcpool.tile([128, KC, Cconv[e]], BF16, tag="xc",
                                  name="xct")
                nc.sync.dma_start(xct[:, 0:2, :], xc_d[e][:, hk, 0:2, :])
                nc.sync.dma_start(xct[:, 2:4, :], xc_d[e][:, hk, 2:4, :])
                xc_tiles[u] = xct

            def conv_unit(u):
                e, hk = divmod(u, HK)
                xct = xc_tiles.pop(u)
                ops = []
                for w0, nw in _windows(Cconv[e]):
                    psc = [None]
                    s = [None]

                    def mm(w0=w0, nw=nw, psc=psc):
                        psc[0] = ps.tile([128, TCH], F32,
                                         tag="pg" if u % 2 == 0 else "pu",
                                         name="psc")
                        for j in range(KC):
                            nc.tensor.matmul(
                                psc[0][:, 0:nw], diag_sb[:, e, hk, j, :],
                                xct[:, j, w0 : w0 + nw],
                                start=(j == 0), stop=(j == KC - 1))

                    def silu(nw=nw, psc=psc, s=s):
                        s[0] = spool.tile([128, TCH], BF16, tag="sc",
                                          name="sconv")
                        nc.scalar.activation(
                            out=s[0][:, 0:nw], in_=psc[0][:, 0:nw],
                            func=AF.Silu)

                    def mul(w0=w0, nw=nw, s=s):
                        nc.vector.tensor_mul(
                            s[0][:, 0:nw], s[0][:, 0:nw],
                            cgc_sb[e][:, w0 : w0 + nw])

                    def store(w0=w0, nw=nw, s=s):
                        nc.sync.dma_start(
                            yc_t[e][:, hk, w0 : w0 + nw], s[0][:, 0:nw])

                    ops += [mm, silu, mul, store]
                ops.append(lambda: xc_fetch(u + 3))
                return ops

            pending = [
                lambda: nc.sync.dma_start(cgb_sb[0], cgb_d[0][:]),
                lambda: nc.sync.dma_start(cgb_sb[1], cgb_d[1][:]),
                lambda: nc.sync.dma_start(cgc_sb[0], cgc_d[0][:]),
                lambda: nc.sync.dma_start(cgc_sb[1], cgc_d[1][:]),
                lambda: xc_fetch(0),
                lambda: xc_fetch(1),
                lambda: xc_fetch(2),
            ]
            conv_left = list(range(2 * HK))

            nslots = sum(IK + HK for _ in phases)
            nops = len(pending) + 2 * HK * 9
            dn = max(2, -(-nops // max(nslots - 12, 1)))

            slot_idx = [0]

            def drain(k):
                # during warmup only the preamble DMAs drain, so the first
                # conv matmuls (which wait on xc) don't block the PE stream
                slot_idx[0] += 1
                for _ in range(k):
                    if not pending:
                        if not conv_left or slot_idx[0] <= 12:
                            return
                        pending.extend(conv_unit(conv_left.pop(0)))
                    pending.pop(0)()

            for pi, (e, w0, nw) in enumerate(phases):
                # ---- gate/up -> A (feature-major [I, nw]) ----
                a_sb = apool.tile([128, IK, TCH], BF16, tag="a")
                wds = []
                for i in range(IK):
                    wgt = wpool.tile([128, HK, 128], BF16, tag="wg")
                    nc.sync.dma_start(wgt, wgr[e, i])
                    wut = wpool.tile([128, HK, 128], BF16, tag="wu")
                    nc.sync.dma_start(wut, wur[e, i])
                    psg = ps.tile([128, TCH], F32, tag="pg")
                    psu = ps.tile([128, TCH], F32, tag="pu")
                    for kc in range(HK):
                        nc.tensor.matmul(
                            psg[:, 0:nw], wgt[:, kc, :],
                            xg_sb[e][:, kc, w0 : w0 + nw],
                            start=(kc == 0), stop=(kc == HK - 1))
                    for kc in range(HK):
                        nc.tensor.matmul(
                            psu[:, 0:nw], wut[:, kc, :],
                            xg_sb[e][:, kc, w0 : w0 + nw],
                            start=(kc == 0), stop=(kc == HK - 1))
                    sg = sgpool.tile([128, TCH], F32, tag="sg")
                    nc.scalar.activation(
                        out=sg[:, 0:nw], in_=psg[:, 0:nw], func=AF.Silu)
                    nc.vector.tensor_mul(
                        a_sb[:, i, 0:nw], sg[:, 0:nw], psu[:, 0:nw])
                    if i == 8:
                        # mid-phase: prefetch down weights + next xg stream
                        for kc in range(IK):
                            wdt = wdpool.tile([128, HK, 128], BF16, tag="wd",
                                              name="wdt")
                            nc.sync.dma_start(wdt, wdr[e, kc])
                            wds.append(wdt)
                        if pi == 0:
                            xg_load(1)
                    drain(dn)

                # ---- down, feature-major: psum[h-tile, tok] ----
                for hb in range(HK):
                    psd = pd.tile([128, TCH], F32, tag="pd")
                    for kc in range(IK):
                        nc.tensor.matmul(
                            psd[:, 0:nw], wds[kc][:, hb, :],
                            a_sb[:, kc, 0:nw],
                            start=(kc == 0), stop=(kc == IK - 1))
                    yt = ytpool.tile([128, TCH], BF16, tag="yt")
                    nc.vector.tensor_mul(
                        yt[:, 0:nw], psd[:, 0:nw],
                        cgb_sb[e][:, w0 : w0 + nw])
                    if pi == len(phases) - 1:
                        # last phase: split the store for lower tail latency
                        h2 = nw // 2
                        nc.sync.dma_start(
                            yf_t[e][:, hb, w0 : w0 + h2], yt[:, 0:h2])
                        nc.sync.dma_start(
                            yf_t[e][:, hb, w0 + h2 : w0 + nw], yt[:, h2:nw])
                    else:
                        nc.sync.dma_start(
                            yf_t[e][:, hb, w0 : w0 + nw], yt[:, 0:nw])
                    drain(dn)
            # flush any remaining conv work
            while pending or conv_left:
                drain(16)
    return legalize_waits(nc)


def _bf16(a):
    return np.asarray(a).astype(ml_dtypes.bfloat16)


def build_in_maps(x, top_k_indices, norm_weights, mlp_gate, mlp_up, mlp_down, conv_w):
    NT = B * S
    xflat = np.asarray(x, dtype=np.float32).reshape(NT, H)
    xflat_b = _bf16(xflat)
    idxflat = np.asarray(top_k_indices).reshape(NT, KTOP)
    nwflat = np.asarray(norm_weights, dtype=np.float32).reshape(NT, KTOP)

    # combined per-expert coefficients, global
    ce = np.zeros((NT, 4), dtype=np.float32)
    rows = np.arange(NT)
    for k in range(KTOP):
        np.add.at(ce, (rows, idxflat[:, k]), nwflat[:, k])

    # globally balanced routing: split every expert's token list evenly
    lists, Cs = [], []
    for e in range(4):
        glst = np.nonzero(ce[:, e] != 0.0)[0]
        parts = np.array_split(glst, NCORES)
        lists.append(parts)
        Cs.append(max(len(p) for p in parts))
    _ROUTE["C"] = Cs
    _ROUTE["lists"] = lists

    # weights, repacked so every DMA tile is contiguous per partition
    wgr = np.ascontiguousarray(
        _bf16(mlp_gate).reshape(2, HK, 128, IK, 128).transpose(0, 3, 2, 1, 4))
    wur = np.ascontiguousarray(
        _bf16(mlp_up).reshape(2, HK, 128, IK, 128).transpose(0, 3, 2, 1, 4))
    wdr = np.ascontiguousarray(_bf16(mlp_down).reshape(2, IK, 128, HK, 128))
    cw = np.asarray(conv_w, dtype=np.float32).reshape(2, HK, 128, KC)
    cwp = np.ascontiguousarray(cw.transpose(2, 0, 1, 3))  # [128, 2, HK, KC]

    def fm_pack(cols_bf16, Cx):
        """[n, H] bf16 -> [128, HK, Cx] zero-padded feature-major."""
        n = cols_bf16.shape[0]
        arr = np.zeros((H, Cx), dtype=ml_dtypes.bfloat16)
        arr[:, :n] = cols_bf16.T
        return np.ascontiguousarray(arr.reshape(HK, 128, Cx).transpose(1, 0, 2))

    def bcast_row(vals, Cx):
        v = np.zeros(Cx, dtype=np.float32)
        v[: len(vals)] = vals
        return np.ascontiguousarray(
            np.broadcast_to(v[None, :], (128, Cx))).astype(ml_dtypes.bfloat16)

    in_maps = []
    for i in range(NCORES):
        im = {"wgr": wgr, "wur": wur, "wdr": wdr, "cwp": cwp}
        for e in range(2):
            lst = lists[e][i]
            im[f"xg{e}"] = fm_pack(xflat_b[lst], Cs[e])
            im[f"cgb{e}"] = bcast_row(ce[lst, e], Cs[e])
        for e in range(2):
            lst = lists[2 + e][i]
            Cx = Cs[2 + e]
            n = len(lst)
            s_in_seq = lst % S
            # taps: xq[j, c] = x[lst[c]+j-3] or 0 at sequence starts
            xc = np.zeros((128, HK, KC, Cx), dtype=ml_dtypes.bfloat16)
            for j in range(KC):
                src = lst + j - (KC - 1)
                valid = (s_in_seq + j - (KC - 1)) >= 0
                cols = np.where(valid[:, None], xflat_b[src * valid], 0)
                xc[:, :, j, :n] = cols.astype(ml_dtypes.bfloat16).T.reshape(
                    HK, 128, n).transpose(1, 0, 2)
            im[f"xc{e}"] = np.ascontiguousarray(xc)
            im[f"cgc{e}"] = bcast_row(ce[lst, 2 + e], Cx)
        in_maps.append(im)
    return in_maps


def assemble(results):
    lists = _ROUTE["lists"]
    out = np.zeros((B * S, H), dtype=np.float32)
    keys = ["yf0", "yf1", "yc0", "yc1"]
    for i, r in enumerate(results):
        for e in range(4):
            lst = lists[e][i]
            n = len(lst)
            yv = np.asarray(r[keys[e]], dtype=np.float32)  # [H, C_e]
            out[lst] += yv[:, :n].T
    return out.reshape(B, S, H)


def kernel(x, top_k_indices, norm_weights, mlp_gate, mlp_up, mlp_down, conv_w):
    in_maps = build_in_maps(
        x, top_k_indices, norm_weights, mlp_gate, mlp_up, mlp_down, conv_w
    )
    nc = build_nc()
    res = run_bass_kernel_spmd(nc, in_maps, core_ids=list(range(NCORES)))
    return assemble(res.results)
